# revision 32
# baseline (speedup 1.0000x reference)
"""Trainium2 Bass kernel for nn_CLUBCategorical (CLUB categorical loss).

Reference computation:
    h      = relu(x @ W1 + b1)              [N, H]
    logits = h @ W2 + b2                    [N, Y]
    logp   = log_softmax(logits, -1)        [N, Y]
    out[i] = logp[i, y_i] - mean_j logp[i, y_j]

Algebraic reduction: the log-softmax normalizer cancels between the
positive and negative terms, and with c[y] = histogram(y_idx),
u = (W2 @ c)/N, beta = (b2 @ c)/N:

    out[i] = h[i,:] @ (W2[:, y_i] - u) + (b2[y_i] - beta)

so the dense phase-2 matmul h @ W2 collapses to a per-row dot product
against a host-gathered matrix GT[k, i] = W2[k, y_i] - u[k]. On device:

    phase 1: hT[m] [128h, 1024r] = relu(W1[:,m].T @ xT + b1)   (PE + ACT)
    phase 2: acc  += hT[m] (.) GT[m]   elementwise, DVE
             out   = ones.T @ acc      (2 ones-matmuls, PE)
    host:    out  += b2[y] - beta

Sharding: data-parallel over N; each core takes 1024 rows with full W1
and its own gathered GT. No collectives. All big operands travel in
bf16 (PE runs bf16 at the same 1 col/cycle as fp32r, so this halves
HBM traffic at no PE cost; tolerance is 2e-2, bf16 lands ~5e-3).
DMA split: x + second-half GT on the sync HWDGE queue, W1 + first-half
GT on the scalar queue, b1 on the gpsimd SWDGE. Warmup matmuls on a
memset tile cover the DMA lead-in and start the PE clock ramp early.
"""

import numpy as np

N, X_DIM, Y_DIM, HIDDEN = 8192, 512, 512, 1024
N_CORES = 8
N_LOC = N // N_CORES          # 1024 rows per core
KX = X_DIM // 128             # 4  k-chunks, phase 1
MH = HIDDEN // 128            # 8  hidden chunks
RG = N_LOC // 512             # 2  row groups of 512

_NC_CACHE = {}


def _build(nc_cls, mybir, tile):
    mdt = mybir.dt
    f32 = mdt.float32
    F32R = mdt.float32r
    BF16 = mdt.bfloat16
    FP16 = mdt.float16
    AF = mybir.ActivationFunctionType
    OP = mybir.AluOpType

    nc = nc_cls("TRN2", target_bir_lowering=False, debug=False,
                num_devices=N_CORES)

    # xt[p, rg*2048 + k*512 + r] = x[rg*512 + r, k*128 + p]
    xtD = nc.dram_tensor("xt", [128, RG * KX * 512], BF16,
                         kind="ExternalInput")
    # w1t[p, m*512 + k*128 + c] = W1[k*128 + p, m*128 + c]
    w1D = nc.dram_tensor("w1t", [128, MH * 512], BF16, kind="ExternalInput")
    # gt[p, m*1024 + i] = W2[m*128 + p, y[i]] - u[m*128 + p]
    gtD = nc.dram_tensor("gt", [128, MH * N_LOC], BF16,
                         kind="ExternalInput")
    # b1c[p, m] = b1[m*128 + p]
    cstD = nc.dram_tensor("cst", [128, MH], f32, kind="ExternalInput")
    out = nc.dram_tensor("out", [1, N_LOC], f32, kind="ExternalOutput")

    with tile.TileContext(nc) as tc:
        with (
            tc.tile_pool(name="wgt", bufs=1) as wgt,
            tc.tile_pool(name="hp", bufs=4) as hp,
            tc.tile_pool(name="prp", bufs=4) as prp,
            tc.tile_pool(name="accp", bufs=1) as accp,
            tc.tile_pool(name="osb", bufs=1) as osb,
            tc.tile_pool(name="ps", bufs=1, space="PSUM") as ps,
        ):
            # on-chip constants: warmup source + ones column (no DMA deps;
            # the verifier requires fp32r matmul operands to be written as
            # F32R, hence the memset -> copy). memset on gpsimd: its
            # framework preamble releases ~0.7us before the DVE's.
            wu_f32 = wgt.tile([128, 512], f32, tag="wuf")
            nc.gpsimd.memset(wu_f32[:], 1.0)
            wu_src = wgt.tile([128, 512], F32R, tag="wur")
            nc.vector.tensor_copy(wu_src[:], wu_f32[:])
            wu_op = wu_src[:]
            ones_f = wgt.tile([128, 1], f32, tag="onesf")
            nc.vector.memset(ones_f[:], 1.0)
            ones_t = wgt.tile([128, 1], FP16, tag="ones")
            nc.vector.tensor_copy(ones_t[:], ones_f[:])

            cst_sb = wgt.tile([128, MH], f32, tag="cst")
            nc.gpsimd.dma_start(cst_sb[:], cstD.ap())

            xt_sb = wgt.tile([128, RG * KX * 512], BF16, tag="xt")
            w1_sb = wgt.tile([128, MH * 512], BF16, tag="w1")
            gt_sb = wgt.tile([128, MH * N_LOC], BF16, tag="gt")

            # Early DMA runs ~3x below steady rate (the DMA engines ramp
            # like the PE), so the PE-gating stream leads in small chunks:
            # sync carries x then the GT second half; scalar carries W1 in
            # consumption order then the GT first half.
            nc.sync.dma_start(xt_sb[:, 0:1024], xtD.ap()[:, 0:1024])
            nc.scalar.dma_start(w1_sb[:, 0:512], w1D.ap()[:, 0:512])
            nc.sync.dma_start(xt_sb[:, 1024:2048], xtD.ap()[:, 1024:2048])
            nc.scalar.dma_start(w1_sb[:, 512:1024], w1D.ap()[:, 512:1024])
            nc.sync.dma_start(w1_sb[:, 1024:1536], w1D.ap()[:, 1024:1536])
            nc.scalar.dma_start(w1_sb[:, 1536:2048], w1D.ap()[:, 1536:2048])
            nc.scalar.dma_start(w1_sb[:, 2048:3072], w1D.ap()[:, 2048:3072])
            nc.scalar.dma_start(w1_sb[:, 3072:4096], w1D.ap()[:, 3072:4096])
            nc.sync.dma_start(gt_sb[:, 0:2048], gtD.ap()[:, 0:2048])
            nc.sync.dma_start(gt_sb[:, 2048:4096], gtD.ap()[:, 2048:4096])
            nc.scalar.dma_start(gt_sb[:, 4096:6144], gtD.ap()[:, 4096:6144])
            nc.sync.dma_start(xt_sb[:, 2048:3072], xtD.ap()[:, 2048:3072])
            nc.sync.dma_start(xt_sb[:, 3072:4096], xtD.ap()[:, 3072:4096])
            nc.scalar.dma_start(gt_sb[:, 6144:8192], gtD.ap()[:, 6144:8192])

            def xt_slice(k, rg):
                o = rg * 2048 + k * 512
                return xt_sb[:, o:o + 512]

            def w1_slice(k, m):
                o = m * 512 + k * 128
                return w1_sb[:, o:o + 128]

            def gt_slice(m, rg):
                o = m * N_LOC + rg * 512
                return gt_sb[:, o:o + 512]


            # warmup: keep the PE busy during the DMA lead-in so the
            # clock ramp starts as early as possible
            wu = ps.tile([128, 512], f32, tag="psum", bufs=6, name="wu")
            for _ in range(5):
                nc.tensor.matmul(wu[:], wu_op[:, 0:128], wu_op[:],
                                 start=True, stop=True)

            acc = [accp.tile([128, 512], FP16, tag=f"acc{rg}",
                             name=f"acc{rg}") for rg in range(RG)]

            def chunk(m, rg):
                psum = ps.tile([128, 512], f32, tag="psum", bufs=6,
                               name=f"p_{rg}_{m}")
                for k in range(KX):
                    nc.tensor.matmul(psum[:], w1_slice(k, m),
                                     xt_slice(k, rg),
                                     start=(k == 0), stop=(k == KX - 1))
                h = hp.tile([128, 512], BF16, tag="h", name=f"h_{rg}_{m}")
                nc.scalar.activation(h[:], psum[:], AF.Relu,
                                     bias=cst_sb[:, m:m + 1])
                # the whole multiply-accumulate runs in 16-bit on the DVE
                # (byte-bound: fp16 ops are 420ns vs fp32's 687 on
                # [128,512]), so it keeps up with the 908ns PE chunk
                # cadence; fp16's 11-bit mantissa keeps the accumulation
                # rounding negligible (bf16's 8 bits would not)
                if m == 0:
                    nc.vector.tensor_tensor(
                        acc[rg][:], h[:], gt_slice(m, rg), OP.mult)
                else:
                    prod = prp.tile([128, 512], FP16, tag="pr",
                                    name=f"pr_{rg}_{m}")
                    nc.vector.tensor_tensor(
                        prod[:], h[:], gt_slice(m, rg), OP.mult)
                    nc.vector.tensor_tensor(
                        acc[rg][:], acc[rg][:], prod[:], OP.add)

            o = osb.tile([1, N_LOC], f32, tag="o")
            pout = [ps.tile([1, 512], f32, tag=f"po{rg}", bufs=1,
                            name=f"po_{rg}") for rg in range(RG)]

            for m in range(MH):
                chunk(m, 0)
            for m in range(5):
                chunk(m, 1)
            # rg0's reduction: emitted 5 chunks into rg1 so the PE never
            # waits on the tail of rg0's DVE accumulation chain. Its copy
            # is deferred to the end (ACT runs it right after the last
            # relu) so the Scalar relu stream is never interrupted.
            nc.tensor.matmul(pout[0][:], ones_t[:], acc[0][:],
                             start=True, stop=True)
            for m in range(5, MH):
                chunk(m, 1)
            nc.scalar.activation(o[:, 0:512], pout[0][:], AF.Copy)
            nc.sync.dma_start(out.ap()[:, 0:512], o[:, 0:512])
            nc.tensor.matmul(pout[1][:], ones_t[:], acc[1][:],
                             start=True, stop=True)
            nc.vector.tensor_copy(o[:, 512:1024], pout[1][:])
            nc.sync.dma_start(out.ap()[:, 512:1024], o[:, 512:1024])

    nc.compile()
    return nc


def _get_nc():
    if "nc" not in _NC_CACHE:
        import concourse.bacc as bacc
        import concourse.mybir as mybir
        from concourse import tile
        _NC_CACHE["nc"] = _build(bacc.Bacc, mybir, tile)
    return _NC_CACHE["nc"]


def kernel(x_samples, y_idx, W1, b1, W2, b2):
    import ml_dtypes
    from concourse.bass_utils import run_bass_kernel_spmd

    bf16 = ml_dtypes.bfloat16
    x = np.ascontiguousarray(np.asarray(x_samples, dtype=np.float32))
    y = np.asarray(y_idx).astype(np.int64).reshape(-1)
    W1 = np.ascontiguousarray(np.asarray(W1, dtype=np.float32))
    b1 = np.asarray(b1, dtype=np.float32).reshape(-1)
    W2 = np.ascontiguousarray(np.asarray(W2, dtype=np.float32))
    b2 = np.asarray(b2, dtype=np.float32).reshape(-1)

    # host-side algebra: label histogram folds the negative term into u,
    # the bias terms fold into g (added back on host)
    c = np.bincount(y, minlength=Y_DIM).astype(np.float32)
    u = (W2 @ c) / np.float32(N)                                  # [H]
    beta = np.float32(b2 @ c) / np.float32(N)
    g_full = (b2[y] - beta).astype(np.float32)                    # [N]

    # w1t[p, m*512 + k*128 + c] = W1[k*128+p, m*128+c]
    w1_dev = np.ascontiguousarray(
        W1.reshape(KX, 128, MH, 128).transpose(1, 2, 0, 3)
        .reshape(128, MH * 512)).astype(bf16)
    b1c = np.ascontiguousarray(b1.reshape(MH, 128).T)             # [128, 8]
    # gathered, recentered W2 columns: GT[k, i] = W2[k, y_i] - u[k]
    gt_all = (W2[:, y] - u[:, None]).astype(bf16)                 # [H, N]

    in_maps = []
    for mcore in range(N_CORES):
        sl = slice(mcore * N_LOC, (mcore + 1) * N_LOC)
        xt_dev = np.ascontiguousarray(
            x[sl].reshape(RG, 512, KX, 128).transpose(3, 0, 2, 1)
            .reshape(128, RG * KX * 512)).astype(bf16)
        gt_dev = np.ascontiguousarray(
            gt_all[:, sl].reshape(MH, 128, N_LOC).transpose(1, 0, 2)
            .reshape(128, MH * N_LOC))
        in_maps.append({"xt": xt_dev, "w1t": w1_dev, "gt": gt_dev,
                        "cst": b1c})

    nc = _get_nc()
    res = run_bass_kernel_spmd(nc, in_maps, core_ids=list(range(N_CORES)))
    return np.concatenate(
        [res.results[m]["out"].reshape(-1) + g_full[m * N_LOC:(m + 1) * N_LOC]
         for m in range(N_CORES)]).astype(np.float32)
